# revision 44
# baseline (speedup 1.0000x reference)
"""CGCNN forward on 8 Trainium2 NeuronCores (Bass/Tile) — v1 redesign.

Strategy (edge parallelism, dst-sorted, h-table AllGather):
  - node rows sharded: core c owns nodes [c*2500, (c+1)*2500), relabeled by
    degree-snake binning into (core, tile, pos); padded to 20 tiles of 128.
  - The POST-BN node state h is AllGathered in fp8 as TWO half-tables
    (tiles 0-9 -> table A, tiles 10-19 -> table B) of [8*1280, 256] each, so
    edge processing of half A overlaps the AllGather of half B.
  - per edge chunk (128 edges with a common dst tile):
      pre = attog_DR_matmul(att||og -> WeT||afas)    (fp8 DoubleRow)
          + GT_DR_matmul(gathered-src-h -> WjT)      (fp8 DoubleRow)
    where GT comes from a per-(tile,half) dma_gather(transpose=True) of the
    replicated h table (256B rows), giving h features pre-packed for
    DoubleRow; the f-gate half of all weights is negated so pre_f = -a.
  - gates: E = exp(pre) (ACT, PSUM pair-batched); v = ln(1+E_s) (ACT);
    t = 1+E_f (DVE); msg = v/t = sigmoid(a)*softplus(b) (DVE divide).
    Only exp/ln/square/copy are used anywhere -> a single ACT table set,
    zero table reloads (BN rsqrt is computed as exp(-0.5*ln(var+eps))).
  - scatter: per-chunk one-hot^T matmul accumulated in PSUM; h += agg*invdeg.
  - BN stats via ones-mask matmuls -> AllReduce [1,512]; affine applied
    per tile; fp8 h snapshot written to the next layer's AG bounces.
  - readout: graph mean-pool one-hot matmul -> AllReduce [64,256] ->
    softplus -> @W_fc -> softplus (replicated).
"""
import sys

for _p in ("/opt/trn_rl_repo", "/root/.axon_site/_ro/trn_rl_repo"):
    if _p not in sys.path:
        sys.path.insert(0, _p)

import numpy as np
import ml_dtypes

BF16 = ml_dtypes.bfloat16

# problem constants (hardcoded per contest rules)
N = 20000
E = 200000
H = 256
IN_NODE = 256
IN_EDGE = 128
NG = 64
L = 2
BN_EPS = 1e-5

NCORES = 8
P = 128
NSH = N // NCORES          # 2500 real nodes per core
NTILES = (NSH + P - 1) // P  # 20
NPAD = NTILES * P          # 2560
HT = NTILES // 2           # 10 tiles per half
HNPAD = HT * P             # 1280 rows per half per core
HROWS = HNPAD * NCORES     # 10240 rows per half table
H2 = 2 * H                 # 512


def _fp8(x):
    from concourse import mybir
    return np.asarray(x).astype(mybir.dt.np(mybir.dt.float8e4))


def _prep(x, edge_index, edge_attr, batch, W_emb, b_emb, W_edge, b_edge,
          Wf, bf, Ws, bs, gamma, beta, W_fc, b_fc):
    """Host-side sharding prep. Returns (static_cfg, shared_inputs, per_core_inputs)."""
    x = np.asarray(x, np.float32)
    ei = np.asarray(edge_index).astype(np.int64)
    ea = np.asarray(edge_attr, np.float32)
    batch = np.asarray(batch).astype(np.int64)

    src, dst = ei[0], ei[1]

    # Node relabeling: deal degree-sorted nodes snake-wise across all
    # (core, tile) bins so per-tile edge counts are balanced across cores.
    NBINS = NCORES * NTILES
    deg_n = np.bincount(dst, minlength=N)
    order_nodes = np.argsort(-deg_n, kind="stable")
    ii = np.arange(N)
    rr = ii // NBINS
    pp = ii % NBINS
    bb = np.where(rr % 2 == 0, pp, NBINS - 1 - pp)
    core_n = np.empty(N, np.int64)
    tile_n = np.empty(N, np.int64)
    pos_n = np.empty(N, np.int64)
    core_n[order_nodes] = bb // NTILES
    tile_n[order_nodes] = bb % NTILES
    pos_n[order_nodes] = rr
    assert pos_n.max() < P

    # src row in the half tables: half = tile >= HT
    half_n = (tile_n >= HT).astype(np.int64)
    hrow_n = core_n * HNPAD + (tile_n % HT) * P + pos_n
    assert hrow_n.max() < HROWS < 32768

    core_of = core_n[dst]
    tile_of = tile_n[dst]
    pos_in_tile = pos_n[dst]
    half_of = half_n[src]

    # per (core, tile): A-edge count and total count. Chunks are packed
    # A-edges-first; chunk roles (all-A / mixed / all-B) are shared across
    # cores: qa = chunks all-A on every core, qb = first chunk all-B on
    # every core. Mixed chunks get TWO j-matmuls (tables A and B) with a
    # reserved zero row (ZROW) filling the other-half positions.
    cntA = np.zeros((NCORES, NTILES), np.int64)
    cntT = np.zeros((NCORES, NTILES), np.int64)
    np.add.at(cntT, (core_of, tile_of), 1)
    np.add.at(cntA, (core_of[half_of == 0], tile_of[half_of == 0]), 1)
    KT = np.maximum(np.ceil(cntT.max(axis=0) / P).astype(np.int64), 1)
    QA = (cntA.min(axis=0) // P).astype(np.int64)
    QB = np.ceil(cntA.max(axis=0) / P).astype(np.int64)
    QB = np.minimum(np.maximum(QB, QA), KT)
    base = np.concatenate([[0], np.cumsum(KT)])[:-1]
    NCHUNK = int(KT.sum())
    # idx16 column layout per tile: [SA(qa) | SMA(qb-qa) | SBB(KT-qa)]
    seg_len = QA + (QB - QA) + (KT - QA)
    segbase = np.concatenate([[0], np.cumsum(seg_len)])[:-1]
    NIDX = int(seg_len.sum())
    ZROW = 127  # (core0, first tile of half, pos 127) is padding on every core

    deg = np.bincount(dst, minlength=N).astype(np.float32)
    inv_deg_full = 1.0 / np.maximum(deg, 1.0)

    gsz = np.bincount(batch, minlength=NG).astype(np.float32)
    inv_gsz = (1.0 / np.maximum(gsz, 1.0)).astype(np.float32).reshape(NG, 1)

    order = np.lexsort((tile_of, core_of))

    per_core = []
    for c in range(NCORES):
        sel = order[core_of[order] == c]
        t_sel = tile_of[sel]

        attog = np.zeros((P, NCHUNK, 3, P), np.float32)
        idx16 = np.zeros((16, NIDX * 8), np.int16)

        for t in range(NTILES):
            e_tile = sel[t_sel == t]
            # A-edges first, then B-edges
            e_t = np.concatenate([e_tile[half_of[e_tile] == 0],
                                  e_tile[half_of[e_tile] == 1]])
            n_t = e_t.size
            kt = int(KT[t])
            qa, qb = int(QA[t]), int(QB[t])
            assert n_t <= kt * P
            b0 = int(base[t])
            j = np.arange(n_t)
            q = j // P
            e_in = j % P
            n_in = pos_in_tile[e_t]
            attog[:128, b0 + q, 0, e_in] = ea[e_t].T       # attr columns
            attog[n_in, b0 + q, 1, e_in] = 1.0             # dst one-hot
            attog[e_in, b0 + q, 2, n_in] = 1.0             # scatter one-hot
            # column tables: colA[q, m] = table-A row of the edge at column m
            # (ZROW otherwise); colB for table B over chunks [qa, kt)
            colA = np.full((kt, P), ZROW, np.int16)
            colB = np.full((kt, P), ZROW, np.int16)
            isA = half_of[e_t] == 0
            colA[q[isA], e_in[isA]] = hrow_n[src[e_t[isA]]].astype(np.int16)
            colB[q[~isA], e_in[~isA]] = hrow_n[src[e_t[~isA]]].astype(np.int16)
            # gather lists with within-chunk position reversal (k = 127-m)
            sb0 = int(segbase[t])

            def put(seg_off, cols):
                flat = cols[:, ::-1].reshape(-1).copy()
                idx16[:, (sb0 + seg_off) * 8:
                      (sb0 + seg_off + cols.shape[0]) * 8] = \
                    flat.reshape(-1, 16).T

            if qa > 0:
                put(0, colA[:qa])                       # SA
            if qb > qa:
                put(qa, colA[qa:qb])                    # SMA
            if kt > qa:
                put(qa + (qb - qa), colB[qa:])          # SBB

        mine = np.nonzero(core_n == c)[0]
        tl, ps = tile_n[mine], pos_n[mine]
        xs = np.zeros((NPAD, IN_NODE), np.float32)
        xs[tl * P + ps] = x[mine]
        xT = np.ascontiguousarray(
            xs.T.reshape(2, P, NPAD).transpose(1, 0, 2)).astype(BF16)

        invdeg = np.zeros((P, NTILES), np.float32)
        invdeg[ps, tl] = inv_deg_full[mine]

        onesmask = np.zeros((P, NTILES), np.float32)
        onesmask[ps, tl] = 1.0

        oggraph = np.zeros((P, NTILES, NG), np.float32)
        oggraph[ps, tl, batch[mine]] = 1.0

        per_core.append(dict(
            xT=xT,
            attog=_fp8(attog),
            idx16=np.ascontiguousarray(np.tile(idx16, (8, 1))),
            invdeg=invdeg,
            onesmask=onesmask,
            oggraph=oggraph,
            invgsz=inv_gsz,
        ))

    # shared (replicated) weights
    Wf = np.asarray(Wf, np.float32)
    Ws = np.asarray(Ws, np.float32)
    bf = np.asarray(bf, np.float32)
    bs = np.asarray(bs, np.float32)
    W_emb = np.asarray(W_emb, np.float32)
    W_edge = np.asarray(W_edge, np.float32)
    b_edge = np.asarray(b_edge, np.float32)
    b_emb = np.asarray(b_emb, np.float32)

    def cat_neg(Wl_f, Wl_s):
        # [256, 512]: f-half negated so pre_f = -a (sigmoid via 1/(1+e^pre_f))
        return np.concatenate([-Wl_f.T, Wl_s.T], axis=1)

    shared = dict(
        W_embT=np.ascontiguousarray(
            W_emb.T.reshape(2, P, H).transpose(1, 0, 2)).astype(BF16),
        b_embrow=np.asarray(b_emb, np.float32).reshape(1, H),
        W_fcT=np.ascontiguousarray(
            np.asarray(W_fc, np.float32).T.reshape(2, P, H).transpose(1, 0, 2)
        ).astype(BF16),
        b_fcrow=np.asarray(b_fc, np.float32).reshape(1, H),
        onescol=np.ones((1, P), np.float32),
        gamma=np.asarray(gamma, np.float32).reshape(L, 1, H),
        beta=np.asarray(beta, np.float32).reshape(L, 1, H),
    )
    for l in range(L):
        wi = cat_neg(Wf[l][:, :H], Ws[l][:, :H])           # [256, 512]
        wj = cat_neg(Wf[l][:, H:2 * H], Ws[l][:, H:2 * H])  # [256, 512]
        # WiT: block-packed [p, s, o] = wi[s*128+p, o], bf16 (afas projection)
        shared[f"WiT_{l}"] = np.ascontiguousarray(
            wi.reshape(2, P, H2).transpose(1, 0, 2)).astype(BF16)
        # WjT8: interleave-packed [p, s, o] = wj[2p+s, o]
        shared[f"WjT8_{l}"] = _fp8(
            np.ascontiguousarray(wj.reshape(P, 2, H2)))
        # edge projection composed with the attr embedding
        shared[f"WeT8_{l}"] = _fp8(np.concatenate([
            -(Wf[l][:, 2 * H:].astype(np.float64) @ W_edge.astype(np.float64)).T,
            (Ws[l][:, 2 * H:].astype(np.float64) @ W_edge.astype(np.float64)).T,
        ], axis=1))  # [128 in_edge, 512]
        brow = np.concatenate([
            -(bf[l] + Wf[l][:, 2 * H:] @ b_edge),
            bs[l] + Ws[l][:, 2 * H:] @ b_edge,
        ]).astype(np.float32).reshape(1, H2)
        shared[f"brow{l}"] = brow

    cfg = dict(KT=[int(k) for k in KT],
               QA=[int(k) for k in QA],
               QB=[int(k) for k in QB],
               base=[int(b) for b in base],
               segbase=[int(b) for b in segbase],
               NCHUNK=NCHUNK, NIDX=NIDX)
    return cfg, shared, per_core


def _patch_act_tables():
    """Pin Exp/Ln to natural_log_exp_and_others so the greedy chooser uses a
    single table for the whole program (we only use exp/ln/square/copy)."""
    import concourse.bacc as bacc_mod
    from concourse import mybir
    from concourse.hw_specs import get_activation_tables as _orig_gat
    if getattr(bacc_mod, "_act_tables_patched", False):
        return
    AF = mybir.ActivationFunctionType

    def _patched(arch):
        tabs = _orig_gat(arch)
        for name, fns in tabs.items():
            if name != "natural_log_exp_and_others":
                fns.discard(AF.Exp)
                fns.discard(AF.Ln)
        return tabs

    bacc_mod.get_activation_tables = _patched
    bacc_mod._act_tables_patched = True


def _build(cfg, reps=1, sim=False):
    """Build the Bass program (same for all cores)."""
    from concourse import bass, bacc, tile, mybir
    from concourse.masks import make_identity
    _patch_act_tables()

    KT = cfg["KT"]
    QA = cfg["QA"]
    QB = cfg["QB"]
    base = cfg["base"]
    segbase = cfg["segbase"]
    NCHUNK = cfg["NCHUNK"]
    NIDX = cfg["NIDX"]
    KTMAX = max(KT)
    fp32 = mybir.dt.float32
    bf16 = mybir.dt.bfloat16
    fp8 = mybir.dt.float8e4
    i16 = mybir.dt.int16
    AF = mybir.ActivationFunctionType
    OP = mybir.AluOpType
    DR = mybir.MatmulPerfMode.DoubleRow
    DRI = mybir.MatmulPerfMode.DoubleRowSwInterleave

    nc = bacc.Bacc("TRN2", target_bir_lowering=False, debug=False,
                   num_devices=1 if sim else NCORES)

    def din(name, shape, dt):
        return nc.dram_tensor(name, list(shape), dt, kind="ExternalInput").ap()

    # per-core inputs
    xT = din("xT", [P, 2, NPAD], bf16)
    attog = din("attog", [P, NCHUNK, 3, P], fp8)
    idx16 = din("idx16", [P, NIDX * 8], i16)
    invdeg = din("invdeg", [P, NTILES], fp32)
    onesmask = din("onesmask", [P, NTILES], fp32)
    oggraph = din("oggraph", [P, NTILES, NG], fp32)
    invgsz = din("invgsz", [NG, 1], fp32)
    # shared weights
    W_embT = din("W_embT", [P, 2, H], bf16)
    b_embrow = din("b_embrow", [1, H], fp32)
    W_fcT = din("W_fcT", [P, 2, H], bf16)
    b_fcrow = din("b_fcrow", [1, H], fp32)
    onescol = din("onescol", [1, P], fp32)
    gamma = din("gamma", [L, 1, H], fp32)
    beta = din("beta", [L, 1, H], fp32)
    WiT = [din(f"WiT_{l}", [P, 2, H2], bf16) for l in range(L)]
    WjT8 = [din(f"WjT8_{l}", [P, 2, H2], fp8) for l in range(L)]
    WeT8 = [din(f"WeT8_{l}", [IN_EDGE, H2], fp8) for l in range(L)]
    brow = [din(f"brow{l}", [1, H2], fp32) for l in range(L)]

    out = nc.dram_tensor("out", [NG, H], fp32, kind="ExternalOutput").ap()

    groups = [list(range(NCORES))]

    with tile.TileContext(nc) as tc:
        with tc.tile_pool(name="const", bufs=1) as const, \
             tc.tile_pool(name="state", bufs=1) as state, \
             tc.tile_pool(name="stream", bufs=3) as stream, \
             tc.tile_pool(name="work", bufs=3) as work, \
             tc.tile_pool(name="pre_ps", bufs=2, space="PSUM") as pre_pool, \
             tc.tile_pool(name="agg_ps", bufs=1, space="PSUM") as agg_pool, \
             tc.tile_pool(name="stat_ps", bufs=1, space="PSUM") as stat_pool, \
             tc.tile_pool(name="misc_ps", bufs=2, space="PSUM") as misc_pool, \
             tc.tile_pool(name="dram", bufs=1, space="DRAM") as dram:

            def misc_ps(shape, name):
                return misc_pool.tile(shape, fp32, tag="mps", name=name)

            # ---------- resident SBUF constants ----------
            def load_const(ap, dt=None, name=None):
                t = const.tile(list(ap.shape), dt or ap.dtype, name=name)
                nc.sync.dma_start(t[:], ap[:])
                return t

            W_embT_sb = load_const(W_embT, name="W_embT_sb")
            b_embrow_sb = load_const(b_embrow, name="b_embrow_sb")
            W_fcT_sb = load_const(W_fcT, name="W_fcT_sb")
            b_fcrow_sb = load_const(b_fcrow, name="b_fcrow_sb")
            onescol_sb = load_const(onescol, name="onescol_sb")
            gamma_sb = []
            beta_sb = []
            for l in range(L):
                gt = const.tile([1, H], fp32, name=f"gamma_sb{l}")
                nc.sync.dma_start(gt[:], gamma[l, :, :])
                gamma_sb.append(gt)
                bt_ = const.tile([1, H], fp32, name=f"beta_sb{l}")
                nc.sync.dma_start(bt_[:], beta[l, :, :])
                beta_sb.append(bt_)
            invdeg_sb = load_const(invdeg, name="invdeg_sb")
            onesmask_sb = load_const(onesmask, name="onesmask_sb")
            oggraph_sb = load_const(oggraph, name="oggraph_sb")
            invgsz_sb = load_const(invgsz, name="invgsz_sb")
            idx16_sb = load_const(idx16, name="idx16_sb")
            WiT_sb = [load_const(WiT[l], name=f"WiT_sb{l}") for l in range(L)]
            WjT8_sb = [load_const(WjT8[l], name=f"WjT8_sb{l}") for l in range(L)]
            WeT8_sb = [load_const(WeT8[l], name=f"WeT8_sb{l}") for l in range(L)]
            brow_sb = [load_const(brow[l], name=f"brow_sb{l}") for l in range(L)]

            ident = const.tile([P, P], fp32, name="ident")
            make_identity(nc, ident[:])

            zeros_c = const.tile([P, 1], fp32, name="zeros_c")
            nc.vector.memset(zeros_c[:], 0.0)
            nc.const_aps.aps[(fp32, 0.0)] = zeros_c[:]
            eps_c = const.tile([P, 1], fp32, name="eps_c")
            nc.vector.memset(eps_c[:], BN_EPS)
            nc.const_aps.aps[(fp32, BN_EPS)] = eps_c[:]
            ones_c = const.tile([P, 1], fp32, name="ones_c")
            nc.vector.memset(ones_c[:], 1.0)
            nc.const_aps.aps[(fp32, 1.0)] = ones_c[:]

            # persistent state
            h_sb = state.tile([P, NTILES, H], fp32, name="h_sb")
            # rhs for the fused att||og matmul: slot0 = WeT, slot1 = afas_t
            afas_sb = state.tile([P, NTILES, 2, H2], fp8, name="afas_sb")
            h8_sb = state.tile([P, NTILES, H], fp8, name="h8_sb")
            nc.vector.memset(h8_sb[:], 0.0)

            for _rep in range(reps):
                hb = [[dram.tile([HNPAD, H], fp8, name=f"hb{l}_{hh}_{_rep}")
                       for hh in range(2)] for l in range(L)]
                hfull = [[dram.tile([HROWS, H], fp8, addr_space="Shared",
                                    name=f"hfull{l}_{hh}_{_rep}")
                          for hh in range(2)] for l in range(L)]
                stats_bounce = [dram.tile([1, H2], fp32, name=f"stats_bounce{l}_{_rep}")
                                for l in range(L)]
                stats_full = [dram.tile([1, H2], fp32, addr_space="Shared",
                                        name=f"stats_full{l}_{_rep}") for l in range(L)]
                g_bounce = dram.tile([NG, H], fp32, name=f"g_bounce{_rep}")
                g_full = dram.tile([NG, H], fp32, addr_space="Shared", name=f"g_full{_rep}")

                def snapshot_h8(t):
                    # fp8 snapshot of h tile (only real rows; padding
                    # partitions >=125 stay zero so ZROW=127 is a zero row)
                    nc.vector.tensor_copy(h8_sb[0:125, t, :], h_sb[0:125, t, :])

                def bounce_half(l, hh):
                    # h8_sb[:, t, :] rows (p) map to table row (t%HT)*P + p;
                    # walk the DRAM side p-major to match the SBUF AP order.
                    # Issued as soon as the half's 10 tiles are ready so the
                    # AllGather overlaps the rest of the phase.
                    dst = hb[l][hh]
                    nc.sync.dma_start(
                        dst[:].rearrange("(t p) c -> p t c", p=P),
                        h8_sb[:, hh * HT:(hh + 1) * HT, :])
                    if sim:
                        nc.sync.dma_start(hfull[l][hh][0:HNPAD, :], dst[:])
                    else:
                        nc.gpsimd.collective_compute(
                            "AllGather", OP.bypass, replica_groups=groups,
                            ins=[dst.opt()], outs=[hfull[l][hh].opt()])

                # ---------- phase H0: h = x @ W_emb^T + b_emb ----------
                xT_sb = stream.tile([P, 2, NPAD], bf16, tag="xT_sb",
                                    bufs=1, name="xT_sb")
                nc.sync.dma_start(xT_sb[:], xT[:])
                for t in range(NTILES):
                    hps = misc_ps([P, H], "hps")
                    nc.tensor.matmul(hps[:], xT_sb[:, 0, t * P:(t + 1) * P],
                                     W_embT_sb[:, 0, :], start=True, stop=False)
                    nc.tensor.matmul(hps[:], xT_sb[:, 1, t * P:(t + 1) * P],
                                     W_embT_sb[:, 1, :], start=False, stop=False)
                    nc.tensor.matmul(hps[:], onescol_sb[:1, :],
                                     b_embrow_sb[:1, :], start=False, stop=True)
                    nc.vector.tensor_copy(h_sb[:, t, :], hps[:])
                    snapshot_h8(t)
                    if t == HT - 1:
                        bounce_half(0, 0)
                bounce_half(0, 1)

                # ---------- layers ----------
                for l in range(L):
                    # --- per-tile rhs slot0 = WeT (const within layer) ---
                    for t in range(NTILES):
                        nc.gpsimd.tensor_copy(afas_sb[:, t, 0, :], WeT8_sb[l][:])

                    # --- node projections: afas (i-side) ---
                    for t in range(NTILES):
                        hT = work.tile([P, 2, P], bf16, tag="hT", bufs=2, name="hT")
                        for k in range(2):
                            tps = misc_ps([P, P], "tps")
                            nc.tensor.transpose(
                                tps[:], h_sb[:, t, k * P:(k + 1) * P], ident[:])
                            nc.vector.tensor_copy(hT[:, k, :], tps[:])
                        aps = misc_ps([P, H2], "aps")
                        nc.tensor.matmul(aps[:], hT[:, 0, :], WiT_sb[l][:, 0, :],
                                         start=True, stop=False)
                        nc.tensor.matmul(aps[:], hT[:, 1, :], WiT_sb[l][:, 1, :],
                                         start=False, stop=False)
                        nc.tensor.matmul(aps[:], onescol_sb[:1, :], brow_sb[l][:1, :],
                                         start=False, stop=True)
                        nc.vector.tensor_copy(afas_sb[:, t, 1, :], aps[:])

                    # --- BN stat accumulator: persistent PSUM bank, the
                    # per-tile sum/sumsq matmuls accumulate into it directly
                    # (no DVE adds) ---
                    stats_ps = stat_pool.tile([1, H2], fp32, name="stats_ps")
                    # zero via DVE once; all stats matmuls accumulate with
                    # start=False (two start=True groups sharing one bank
                    # would re-mark the 2KB zero-region and drop partials)
                    nc.vector.memset(stats_ps[:], 0.0)
                    stats_n = [0]

                    # --- edge passes: pass A = all-A chunks (table A only);
                    #     pass B = mixed + all-B chunks (tables A and B) ---
                    def process_chunks(t, c0, ncv, jmms):
                        """Process chunks [c0, c0+ncv) of tile t. jmms maps
                        local chunk index -> list of (gt_flat, byte_off)."""
                        b0 = base[t] + c0
                        att_t = stream.tile([P, ncv, 3, P], fp8, tag="att",
                                            name="att_t", bufs=4,
                                            padded_shape=[P, KTMAX, 3, P])
                        nc.sync.dma_start(att_t[:], attog[:, b0:b0 + ncv, :, :])

                        E_t = work.tile([P, ncv, H2], bf16, tag="E_t",
                                        name="E_t", bufs=2,
                                        padded_shape=[P, KTMAX, H2])
                        for i0 in range(0, ncv, 2):
                            n2 = min(2, ncv - i0)
                            pr = pre_pool.tile([P, 2, H2], fp32, name="pre")
                            for d in range(n2):
                                i = i0 + d
                                nc.tensor.matmul(
                                    pr[:, d, :], att_t[:, i, 0:2, :],
                                    afas_sb[:, t, :, :],
                                    start=True, stop=False, perf_mode=DR)
                                jl = jmms(i)
                                for w, (gflat, off) in enumerate(jl):
                                    nc.tensor.matmul(
                                        pr[:, d, :],
                                        gflat[:, off:off + 2 * P],
                                        WjT8_sb[l][:],
                                        start=False, stop=(w == len(jl) - 1),
                                        perf_mode=DRI)
                            nc.scalar.activation(
                                E_t[:, i0:i0 + n2, :],
                                pr[:, 0:n2, :] if n2 == 2 else pr[:, 0, :],
                                AF.Exp)

                        # u||v = ln(1+E); sigma = exp(-u); msg = sigma * v
                        uv_t = work.tile([P, ncv, H2], bf16, tag="uv_t",
                                         name="uv_t", bufs=2,
                                         padded_shape=[P, KTMAX, H2])
                        nc.scalar.activation(uv_t[:], E_t[:], AF.Ln, bias=1.0)
                        sg_t = work.tile([P, ncv, H], bf16, tag="sg_t",
                                         name="sg_t", bufs=2,
                                         padded_shape=[P, KTMAX, H])
                        nc.scalar.activation(sg_t[:], uv_t[:, :, 0:H],
                                             AF.Exp, scale=-1.0)
                        msg_t = work.tile([P, ncv, H], fp8, tag="msg_t",
                                          name="msg_t", bufs=2,
                                          padded_shape=[P, KTMAX, H])
                        nc.vector.tensor_tensor(out=msg_t[:], in0=sg_t[:],
                                                in1=uv_t[:, :, H:], op=OP.mult)

                        # paired fp8 DoubleRow scatter: two chunks per matmul
                        agg = agg_pool.tile([P, H], fp32, name="agg")
                        for i0 in range(0, ncv, 2):
                            if i0 + 2 <= ncv:
                                nc.tensor.matmul(
                                    agg[:], att_t[:, i0:i0 + 2, 2, :],
                                    msg_t[:, i0:i0 + 2, :],
                                    start=(i0 == 0), stop=(i0 + 2 == ncv),
                                    perf_mode=DR)
                            else:
                                nc.tensor.matmul(
                                    agg[:], att_t[:, i0, 2, :], msg_t[:, i0, :],
                                    start=(i0 == 0), stop=True)
                        nc.vector.scalar_tensor_tensor(
                            out=h_sb[:, t, :], in0=agg[:],
                            scalar=invdeg_sb[:, t:t + 1], in1=h_sb[:, t, :],
                            op0=OP.mult, op1=OP.add)

                    def gather_seg(src_full, col0, nch):
                        gt = stream.tile([P, 2 * KTMAX * P], fp8, tag="gt",
                                         name="gt", bufs=4)
                        gt_ap = gt[:, 0:2 * nch * P].rearrange(
                            "p (a e) -> p a e", a=2)
                        nc.gpsimd.dma_gather(
                            gt_ap, src_full[:],
                            idx16_sb[:, col0 * 8:(col0 + nch) * 8],
                            nch * P, nch * P, H, transpose=True)
                        return gt[:]

                    def tile_stats(t):
                        hsq = work.tile([P, H], fp32, tag="hsq", bufs=3,
                                        name="hsq")
                        nc.vector.tensor_tensor(out=hsq[:], in0=h_sb[:, t, :],
                                                in1=h_sb[:, t, :], op=OP.mult)
                        i = stats_n[0]
                        stats_n[0] += 1
                        nc.tensor.matmul(stats_ps[:, :H],
                                         onesmask_sb[:, t:t + 1],
                                         h_sb[:, t, :], start=False,
                                         stop=(i == NTILES - 1),
                                         skip_group_check=True)
                        nc.tensor.matmul(stats_ps[:, H:],
                                         onesmask_sb[:, t:t + 1],
                                         hsq[:], start=False,
                                         stop=(i == NTILES - 1),
                                         skip_group_check=True)

                    # First SPLIT tiles run their all-A chunks while the
                    # table-B AllGather is in flight; the rest run single-pass
                    # (one gather per table, all chunks, one h update).
                    SPLIT = 2
                    for t in range(SPLIT):
                        qa = QA[t]
                        if qa == 0:
                            continue
                        gA = gather_seg(hfull[l][0], segbase[t], qa)
                        process_chunks(t, 0, qa,
                                       lambda i, gA=gA: [(gA, i * 2 * P)])

                    for t in range(SPLIT, NTILES):
                        qa, qb, kt = QA[t], QB[t], KT[t]
                        nm = qb - qa
                        gA = gather_seg(hfull[l][0], segbase[t], qb) \
                            if qb > 0 else None
                        gB = gather_seg(hfull[l][1],
                                        segbase[t] + qb, kt - qa) \
                            if kt > qa else None

                        def jmms(i, gA=gA, gB=gB, qa=qa, qb=qb):
                            jl = []
                            if i < qb:
                                jl.append((gA, i * 2 * P))
                            if i >= qa:
                                jl.append((gB, (i - qa) * 2 * P))
                            return jl

                        process_chunks(t, 0, kt, jmms)
                        tile_stats(t)

                    # pass B of the split tiles
                    for t in range(SPLIT):
                        qa, qb, kt = QA[t], QB[t], KT[t]
                        nm = qb - qa
                        nb = kt - qa
                        if nb > 0:
                            gM = None
                            if nm > 0:
                                gM = gather_seg(hfull[l][0],
                                                segbase[t] + qa, nm)
                            gB = gather_seg(hfull[l][1],
                                            segbase[t] + qa + nm, nb)

                            def jmms(i, gM=gM, gB=gB, nm=nm):
                                jl = []
                                if i < nm:
                                    jl.append((gM, i * 2 * P))
                                jl.append((gB, i * 2 * P))
                                return jl

                            process_chunks(t, qa, nb, jmms)
                        tile_stats(t)

                    # --- BN: allreduce stats, apply affine ---
                    stats_acc = work.tile([1, H2], fp32, tag="stats_acc",
                                          name="stats_acc")
                    nc.vector.tensor_copy(stats_acc[:], stats_ps[:])
                    nc.sync.dma_start(stats_bounce[l][:], stats_acc[:])
                    if sim:
                        nc.sync.dma_start(stats_full[l][:], stats_bounce[l][:])
                    else:
                        nc.gpsimd.collective_compute(
                            "AllReduce", OP.add, replica_groups=groups,
                            ins=[stats_bounce[l].opt()],
                            outs=[stats_full[l].opt()])
                    statsr = work.tile([1, H2], fp32, tag="small", bufs=1, name="statsr")
                    nc.sync.dma_start(statsr[:], stats_full[l][:])
                    ab = work.tile([1, H2], fp32, tag="small", bufs=1, name="ab")
                    mu = work.tile([1, H], fp32, tag="small2", name="mu")
                    nc.vector.tensor_scalar_mul(mu[:], statsr[:, :H], 1.0 / N)
                    var = work.tile([1, H], fp32, tag="small2", name="var")
                    nc.vector.tensor_scalar_mul(var[:], statsr[:, H:], 1.0 / N)
                    musq = work.tile([1, H], fp32, tag="small2", name="musq")
                    nc.vector.tensor_tensor(out=musq[:], in0=mu[:], in1=mu[:],
                                            op=OP.mult)
                    nc.vector.tensor_tensor(out=var[:], in0=var[:], in1=musq[:],
                                            op=OP.subtract)
                    # A = gamma * exp(-0.5*ln(var+eps))  (no sqrt table needed)
                    lnv = work.tile([1, H], fp32, tag="small2", name="lnv")
                    nc.scalar.activation(lnv[:], var[:], AF.Ln, bias=BN_EPS)
                    rsd = work.tile([1, H], fp32, tag="small2", name="rsd")
                    nc.scalar.activation(rsd[:], lnv[:], AF.Exp, scale=-0.5)
                    nc.vector.tensor_tensor(out=ab[:, :H], in0=rsd[:],
                                            in1=gamma_sb[l][:], op=OP.mult)
                    nc.vector.tensor_tensor(out=ab[:, H:], in0=mu[:],
                                            in1=ab[:, :H], op=OP.mult)
                    nc.vector.tensor_tensor(out=ab[:, H:], in0=beta_sb[l][:],
                                            in1=ab[:, H:], op=OP.subtract)
                    abps = misc_ps([P, H2], "abps")
                    nc.tensor.matmul(abps[:], onescol_sb[:1, :], ab[:1, :],
                                     start=True, stop=True)
                    abb = work.tile([P, H2], fp32, tag="abb", bufs=1, name="abb")
                    nc.vector.tensor_copy(abb[:], abps[:])
                    for t in range(NTILES):
                        nc.vector.tensor_tensor(out=h_sb[:, t, :],
                                                in0=h_sb[:, t, :],
                                                in1=abb[:, :H], op=OP.mult)
                        nc.vector.tensor_tensor(out=h_sb[:, t, :],
                                                in0=h_sb[:, t, :],
                                                in1=abb[:, H:], op=OP.add)
                        if l + 1 < L:
                            snapshot_h8(t)
                            if t == HT - 1:
                                bounce_half(l + 1, 0)
                    if l + 1 < L:
                        bounce_half(l + 1, 1)

                # ---------- readout ----------
                gp = misc_ps([NG, H], "gp")
                for t in range(NTILES):
                    nc.tensor.matmul(gp[:], oggraph_sb[:, t, :], h_sb[:, t, :],
                                     start=(t == 0), stop=(t == NTILES - 1))
                gp_sb = work.tile([NG, H], fp32, tag="gp_sb", bufs=1, name="gp_sb")
                nc.vector.tensor_scalar(out=gp_sb[:], in0=gp[:],
                                        scalar1=invgsz_sb[:, :1], scalar2=None,
                                        op0=mybir.AluOpType.mult)
                nc.sync.dma_start(g_bounce[:], gp_sb[:])
                if sim:
                    nc.sync.dma_start(g_full[:], g_bounce[:])
                else:
                    nc.gpsimd.collective_compute(
                        "AllReduce", OP.add, replica_groups=groups,
                        ins=[g_bounce.opt()], outs=[g_full.opt()])
                gr = work.tile([NG, H], fp32, tag="gr", bufs=1, name="gr")
                nc.sync.dma_start(gr[:], g_full[:])
                ge = work.tile([NG, H], fp32, tag="ge", bufs=1, name="ge")
                nc.scalar.activation(ge[:], gr[:], AF.Exp)
                spg = work.tile([NG, H], fp32, tag="spg", bufs=1, name="spg")
                nc.scalar.activation(spg[:], ge[:], AF.Ln, bias=1.0)
                spgT = work.tile([P, 2, NG], bf16, tag="spgT", bufs=1, name="spgT")
                for k in range(2):
                    tp = misc_ps([P, NG], "tp")
                    nc.tensor.transpose(tp[:], spg[:, k * P:(k + 1) * P],
                                        ident[:NG, :NG])
                    nc.scalar.activation(spgT[:, k, :], tp[:], AF.Copy)
                ops_ = misc_ps([NG, H], "ops_")
                nc.tensor.matmul(ops_[:], spgT[:, 0, :], W_fcT_sb[:, 0, :],
                                 start=True, stop=False)
                nc.tensor.matmul(ops_[:], spgT[:, 1, :], W_fcT_sb[:, 1, :],
                                 start=False, stop=False)
                nc.tensor.matmul(ops_[:], onescol_sb[:1, :NG], b_fcrow_sb[:1, :],
                                 start=False, stop=True)
                oe = work.tile([NG, H], fp32, tag="oe", bufs=1, name="oe")
                nc.scalar.activation(oe[:], ops_[:], AF.Exp)
                out_sb = work.tile([NG, H], fp32, tag="out_sb", bufs=1, name="out_sb")
                nc.scalar.activation(out_sb[:], oe[:], AF.Ln, bias=1.0)
                nc.sync.dma_start(out[:], out_sb[:])

    nc.compile()
    return nc


def kernel(**inputs):
    from concourse import bass_utils

    cfg, shared, per_core = _prep(**inputs)
    nc = _build(cfg)

    in_maps = []
    for c in range(NCORES):
        m = dict(shared)
        m.update(per_core[c])
        in_maps.append(m)

    res = bass_utils.run_bass_kernel_spmd(
        nc, in_maps, core_ids=list(range(NCORES)))
    return np.asarray(res.results[0]["out"], np.float32)


if __name__ == "__main__":
    data = np.load("/root/problem/ref_data.npz")
    inputs = {k: data[k] for k in data.files if k != "expected"}
    got = kernel(**inputs)
    exp = data["expected"]
    err = np.abs(got - exp).max() / max(np.abs(exp).max(), 1e-9)
    fro = np.linalg.norm(got - exp) / np.linalg.norm(exp)
    print("max abs rel err:", err, "fro:", fro)


# revision 45
# speedup vs baseline: 2.1924x; 2.1924x over previous
"""CGCNN forward on 8 Trainium2 NeuronCores (Bass/Tile) — v1 redesign.

Strategy (edge parallelism, dst-sorted, h-table AllGather):
  - node rows sharded: core c owns nodes [c*2500, (c+1)*2500), relabeled by
    degree-snake binning into (core, tile, pos); padded to 20 tiles of 128.
  - The POST-BN node state h is AllGathered in fp8 as TWO half-tables
    (tiles 0-9 -> table A, tiles 10-19 -> table B) of [8*1280, 256] each, so
    edge processing of half A overlaps the AllGather of half B.
  - per edge chunk (128 edges with a common dst tile):
      pre = attog_DR_matmul(att||og -> WeT||afas)    (fp8 DoubleRow)
          + GT_DR_matmul(gathered-src-h -> WjT)      (fp8 DoubleRow)
    where GT comes from a per-(tile,half) dma_gather(transpose=True) of the
    replicated h table (256B rows), giving h features pre-packed for
    DoubleRow; the f-gate half of all weights is negated so pre_f = -a.
  - gates: E = exp(pre) (ACT, PSUM pair-batched); v = ln(1+E_s) (ACT);
    t = 1+E_f (DVE); msg = v/t = sigmoid(a)*softplus(b) (DVE divide).
    Only exp/ln/square/copy are used anywhere -> a single ACT table set,
    zero table reloads (BN rsqrt is computed as exp(-0.5*ln(var+eps))).
  - scatter: per-chunk one-hot^T matmul accumulated in PSUM; h += agg*invdeg.
  - BN stats via ones-mask matmuls -> AllReduce [1,512]; affine applied
    per tile; fp8 h snapshot written to the next layer's AG bounces.
  - readout: graph mean-pool one-hot matmul -> AllReduce [64,256] ->
    softplus -> @W_fc -> softplus (replicated).
"""
import sys

for _p in ("/opt/trn_rl_repo", "/root/.axon_site/_ro/trn_rl_repo"):
    if _p not in sys.path:
        sys.path.insert(0, _p)

import numpy as np
import ml_dtypes

BF16 = ml_dtypes.bfloat16

# problem constants (hardcoded per contest rules)
N = 20000
E = 200000
H = 256
IN_NODE = 256
IN_EDGE = 128
NG = 64
L = 2
BN_EPS = 1e-5

NCORES = 8
P = 128
NSH = N // NCORES          # 2500 real nodes per core
NTILES = (NSH + P - 1) // P  # 20
NPAD = NTILES * P          # 2560
HT = NTILES // 2           # 10 tiles per half
HNPAD = HT * P             # 1280 rows per half per core
HROWS = HNPAD * NCORES     # 10240 rows per half table
H2 = 2 * H                 # 512


def _fp8(x):
    from concourse import mybir
    return np.asarray(x).astype(mybir.dt.np(mybir.dt.float8e4))


def _prep(x, edge_index, edge_attr, batch, W_emb, b_emb, W_edge, b_edge,
          Wf, bf, Ws, bs, gamma, beta, W_fc, b_fc):
    """Host-side sharding prep. Returns (static_cfg, shared_inputs, per_core_inputs)."""
    x = np.asarray(x, np.float32)
    ei = np.asarray(edge_index).astype(np.int64)
    ea = np.asarray(edge_attr, np.float32)
    batch = np.asarray(batch).astype(np.int64)

    src, dst = ei[0], ei[1]

    # Node relabeling: deal degree-sorted nodes snake-wise across all
    # (core, tile) bins so per-tile edge counts are balanced across cores.
    NBINS = NCORES * NTILES
    deg_n = np.bincount(dst, minlength=N)
    order_nodes = np.argsort(-deg_n, kind="stable")
    ii = np.arange(N)
    rr = ii // NBINS
    pp = ii % NBINS
    bb = np.where(rr % 2 == 0, pp, NBINS - 1 - pp)
    core_n = np.empty(N, np.int64)
    tile_n = np.empty(N, np.int64)
    pos_n = np.empty(N, np.int64)
    core_n[order_nodes] = bb // NTILES
    tile_n[order_nodes] = bb % NTILES
    pos_n[order_nodes] = rr
    assert pos_n.max() < P

    # src row in the half tables: half = tile >= HT
    half_n = (tile_n >= HT).astype(np.int64)
    hrow_n = core_n * HNPAD + (tile_n % HT) * P + pos_n
    assert hrow_n.max() < HROWS < 32768

    core_of = core_n[dst]
    tile_of = tile_n[dst]
    pos_in_tile = pos_n[dst]
    half_of = half_n[src]

    # per (core, tile): A-edge count and total count. Chunks are packed
    # A-edges-first; chunk roles (all-A / mixed / all-B) are shared across
    # cores: qa = chunks all-A on every core, qb = first chunk all-B on
    # every core. Mixed chunks get TWO j-matmuls (tables A and B) with a
    # reserved zero row (ZROW) filling the other-half positions.
    cntA = np.zeros((NCORES, NTILES), np.int64)
    cntT = np.zeros((NCORES, NTILES), np.int64)
    np.add.at(cntT, (core_of, tile_of), 1)
    np.add.at(cntA, (core_of[half_of == 0], tile_of[half_of == 0]), 1)
    KT = np.maximum(np.ceil(cntT.max(axis=0) / P).astype(np.int64), 1)
    QA = (cntA.min(axis=0) // P).astype(np.int64)
    QB = np.ceil(cntA.max(axis=0) / P).astype(np.int64)
    QB = np.minimum(np.maximum(QB, QA), KT)
    base = np.concatenate([[0], np.cumsum(KT)])[:-1]
    NCHUNK = int(KT.sum())
    # idx16 column layout per tile: [SA(qa) | SMA(qb-qa) | SBB(KT-qa)]
    seg_len = QA + (QB - QA) + (KT - QA)
    segbase = np.concatenate([[0], np.cumsum(seg_len)])[:-1]
    NIDX = int(seg_len.sum())
    ZROW = 127  # (core0, first tile of half, pos 127) is padding on every core

    deg = np.bincount(dst, minlength=N).astype(np.float32)
    inv_deg_full = 1.0 / np.maximum(deg, 1.0)

    gsz = np.bincount(batch, minlength=NG).astype(np.float32)
    inv_gsz = (1.0 / np.maximum(gsz, 1.0)).astype(np.float32).reshape(NG, 1)

    order = np.lexsort((tile_of, core_of))

    per_core = []
    for c in range(NCORES):
        sel = order[core_of[order] == c]
        t_sel = tile_of[sel]

        attog = np.zeros((P, NCHUNK, 3, P), np.float32)
        idx16 = np.zeros((16, NIDX * 8), np.int16)

        for t in range(NTILES):
            e_tile = sel[t_sel == t]
            # A-edges first, then B-edges
            e_t = np.concatenate([e_tile[half_of[e_tile] == 0],
                                  e_tile[half_of[e_tile] == 1]])
            n_t = e_t.size
            kt = int(KT[t])
            qa, qb = int(QA[t]), int(QB[t])
            assert n_t <= kt * P
            b0 = int(base[t])
            j = np.arange(n_t)
            q = j // P
            e_in = j % P
            n_in = pos_in_tile[e_t]
            attog[:128, b0 + q, 0, e_in] = ea[e_t].T       # attr columns
            attog[n_in, b0 + q, 1, e_in] = 1.0             # dst one-hot
            attog[e_in, b0 + q, 2, n_in] = 1.0             # scatter one-hot
            # column tables: colA[q, m] = table-A row of the edge at column m
            # (ZROW otherwise); colB for table B over chunks [qa, kt)
            colA = np.full((kt, P), ZROW, np.int16)
            colB = np.full((kt, P), ZROW, np.int16)
            isA = half_of[e_t] == 0
            colA[q[isA], e_in[isA]] = hrow_n[src[e_t[isA]]].astype(np.int16)
            colB[q[~isA], e_in[~isA]] = hrow_n[src[e_t[~isA]]].astype(np.int16)
            # gather lists with within-chunk position reversal (k = 127-m)
            sb0 = int(segbase[t])

            def put(seg_off, cols):
                flat = cols[:, ::-1].reshape(-1).copy()
                idx16[:, (sb0 + seg_off) * 8:
                      (sb0 + seg_off + cols.shape[0]) * 8] = \
                    flat.reshape(-1, 16).T

            if qa > 0:
                put(0, colA[:qa])                       # SA
            if qb > qa:
                put(qa, colA[qa:qb])                    # SMA
            if kt > qa:
                put(qa + (qb - qa), colB[qa:])          # SBB

        mine = np.nonzero(core_n == c)[0]
        tl, ps = tile_n[mine], pos_n[mine]
        xs = np.zeros((NPAD, IN_NODE), np.float32)
        xs[tl * P + ps] = x[mine]
        xT = np.ascontiguousarray(
            xs.T.reshape(2, P, NPAD).transpose(1, 0, 2)).astype(BF16)

        invdeg = np.zeros((P, NTILES), np.float32)
        invdeg[ps, tl] = inv_deg_full[mine]

        onesmask = np.zeros((P, NTILES), np.float32)
        onesmask[ps, tl] = 1.0

        oggraph = np.zeros((P, NTILES, NG), np.float32)
        oggraph[ps, tl, batch[mine]] = 1.0

        per_core.append(dict(
            xT=xT,
            attog=_fp8(attog),
            idx16=np.ascontiguousarray(np.tile(idx16, (8, 1))),
            invdeg=invdeg,
            onesmask=onesmask,
            oggraph=oggraph,
            invgsz=inv_gsz,
        ))

    # shared (replicated) weights
    Wf = np.asarray(Wf, np.float32)
    Ws = np.asarray(Ws, np.float32)
    bf = np.asarray(bf, np.float32)
    bs = np.asarray(bs, np.float32)
    W_emb = np.asarray(W_emb, np.float32)
    W_edge = np.asarray(W_edge, np.float32)
    b_edge = np.asarray(b_edge, np.float32)
    b_emb = np.asarray(b_emb, np.float32)

    def cat_neg(Wl_f, Wl_s):
        # [256, 512]: f-half negated so pre_f = -a (sigmoid via 1/(1+e^pre_f))
        return np.concatenate([-Wl_f.T, Wl_s.T], axis=1)

    shared = dict(
        W_embT=np.ascontiguousarray(
            W_emb.T.reshape(2, P, H).transpose(1, 0, 2)).astype(BF16),
        b_embrow=np.asarray(b_emb, np.float32).reshape(1, H),
        W_fcT=np.ascontiguousarray(
            np.asarray(W_fc, np.float32).T.reshape(2, P, H).transpose(1, 0, 2)
        ).astype(BF16),
        b_fcrow=np.asarray(b_fc, np.float32).reshape(1, H),
        onescol=np.ones((1, P), np.float32),
        gamma=np.asarray(gamma, np.float32).reshape(L, 1, H),
        beta=np.asarray(beta, np.float32).reshape(L, 1, H),
    )
    for l in range(L):
        wi = cat_neg(Wf[l][:, :H], Ws[l][:, :H])           # [256, 512]
        wj = cat_neg(Wf[l][:, H:2 * H], Ws[l][:, H:2 * H])  # [256, 512]
        # WiT: block-packed [p, s, o] = wi[s*128+p, o], bf16 (afas projection)
        shared[f"WiT_{l}"] = np.ascontiguousarray(
            wi.reshape(2, P, H2).transpose(1, 0, 2)).astype(BF16)
        # WjT8: interleave-packed [p, s, o] = wj[2p+s, o]
        shared[f"WjT8_{l}"] = _fp8(
            np.ascontiguousarray(wj.reshape(P, 2, H2)))
        # edge projection composed with the attr embedding
        shared[f"WeT8_{l}"] = _fp8(np.concatenate([
            -(Wf[l][:, 2 * H:].astype(np.float64) @ W_edge.astype(np.float64)).T,
            (Ws[l][:, 2 * H:].astype(np.float64) @ W_edge.astype(np.float64)).T,
        ], axis=1))  # [128 in_edge, 512]
        brow = np.concatenate([
            -(bf[l] + Wf[l][:, 2 * H:] @ b_edge),
            bs[l] + Ws[l][:, 2 * H:] @ b_edge,
        ]).astype(np.float32).reshape(1, H2)
        shared[f"brow{l}"] = brow

    cfg = dict(KT=[int(k) for k in KT],
               QA=[int(k) for k in QA],
               QB=[int(k) for k in QB],
               base=[int(b) for b in base],
               segbase=[int(b) for b in segbase],
               NCHUNK=NCHUNK, NIDX=NIDX)
    return cfg, shared, per_core


def _patch_act_tables():
    """Pin Exp/Ln to natural_log_exp_and_others so the greedy chooser uses a
    single table for the whole program (we only use exp/ln/square/copy)."""
    import concourse.bacc as bacc_mod
    from concourse import mybir
    from concourse.hw_specs import get_activation_tables as _orig_gat
    if getattr(bacc_mod, "_act_tables_patched", False):
        return
    AF = mybir.ActivationFunctionType

    def _patched(arch):
        tabs = _orig_gat(arch)
        for name, fns in tabs.items():
            if name != "natural_log_exp_and_others":
                fns.discard(AF.Exp)
                fns.discard(AF.Ln)
        return tabs

    bacc_mod.get_activation_tables = _patched
    bacc_mod._act_tables_patched = True


def _build(cfg, reps=1, sim=False):
    """Build the Bass program (same for all cores)."""
    from concourse import bass, bacc, tile, mybir
    from concourse.masks import make_identity
    _patch_act_tables()

    KT = cfg["KT"]
    QA = cfg["QA"]
    QB = cfg["QB"]
    base = cfg["base"]
    segbase = cfg["segbase"]
    NCHUNK = cfg["NCHUNK"]
    NIDX = cfg["NIDX"]
    KTMAX = max(KT)
    fp32 = mybir.dt.float32
    bf16 = mybir.dt.bfloat16
    fp8 = mybir.dt.float8e4
    i16 = mybir.dt.int16
    AF = mybir.ActivationFunctionType
    OP = mybir.AluOpType
    DR = mybir.MatmulPerfMode.DoubleRow
    DRI = mybir.MatmulPerfMode.DoubleRowSwInterleave

    nc = bacc.Bacc("TRN2", target_bir_lowering=False, debug=False,
                   num_devices=1 if sim else NCORES)

    def din(name, shape, dt):
        return nc.dram_tensor(name, list(shape), dt, kind="ExternalInput").ap()

    # per-core inputs
    xT = din("xT", [P, 2, NPAD], bf16)
    attog = din("attog", [P, NCHUNK, 3, P], fp8)
    idx16 = din("idx16", [P, NIDX * 8], i16)
    invdeg = din("invdeg", [P, NTILES], fp32)
    onesmask = din("onesmask", [P, NTILES], fp32)
    oggraph = din("oggraph", [P, NTILES, NG], fp32)
    invgsz = din("invgsz", [NG, 1], fp32)
    # shared weights
    W_embT = din("W_embT", [P, 2, H], bf16)
    b_embrow = din("b_embrow", [1, H], fp32)
    W_fcT = din("W_fcT", [P, 2, H], bf16)
    b_fcrow = din("b_fcrow", [1, H], fp32)
    onescol = din("onescol", [1, P], fp32)
    gamma = din("gamma", [L, 1, H], fp32)
    beta = din("beta", [L, 1, H], fp32)
    WiT = [din(f"WiT_{l}", [P, 2, H2], bf16) for l in range(L)]
    WjT8 = [din(f"WjT8_{l}", [P, 2, H2], fp8) for l in range(L)]
    WeT8 = [din(f"WeT8_{l}", [IN_EDGE, H2], fp8) for l in range(L)]
    brow = [din(f"brow{l}", [1, H2], fp32) for l in range(L)]

    out = nc.dram_tensor("out", [NG, H], fp32, kind="ExternalOutput").ap()

    groups = [list(range(NCORES))]

    with tile.TileContext(nc) as tc:
        with tc.tile_pool(name="const", bufs=1) as const, \
             tc.tile_pool(name="state", bufs=1) as state, \
             tc.tile_pool(name="stream", bufs=3) as stream, \
             tc.tile_pool(name="work", bufs=3) as work, \
             tc.tile_pool(name="pre_ps", bufs=2, space="PSUM") as pre_pool, \
             tc.tile_pool(name="agg_ps", bufs=1, space="PSUM") as agg_pool, \
             tc.tile_pool(name="stat_ps", bufs=1, space="PSUM") as stat_pool, \
             tc.tile_pool(name="misc_ps", bufs=2, space="PSUM") as misc_pool, \
             tc.tile_pool(name="dram", bufs=1, space="DRAM") as dram:

            def misc_ps(shape, name):
                return misc_pool.tile(shape, fp32, tag="mps", name=name)

            # ---------- resident SBUF constants ----------
            def load_const(ap, dt=None, name=None):
                t = const.tile(list(ap.shape), dt or ap.dtype, name=name)
                nc.sync.dma_start(t[:], ap[:])
                return t

            W_embT_sb = load_const(W_embT, name="W_embT_sb")
            b_embrow_sb = load_const(b_embrow, name="b_embrow_sb")
            W_fcT_sb = load_const(W_fcT, name="W_fcT_sb")
            b_fcrow_sb = load_const(b_fcrow, name="b_fcrow_sb")
            onescol_sb = load_const(onescol, name="onescol_sb")
            gamma_sb = []
            beta_sb = []
            for l in range(L):
                gt = const.tile([1, H], fp32, name=f"gamma_sb{l}")
                nc.sync.dma_start(gt[:], gamma[l, :, :])
                gamma_sb.append(gt)
                bt_ = const.tile([1, H], fp32, name=f"beta_sb{l}")
                nc.sync.dma_start(bt_[:], beta[l, :, :])
                beta_sb.append(bt_)
            invdeg_sb = load_const(invdeg, name="invdeg_sb")
            onesmask_sb = load_const(onesmask, name="onesmask_sb")
            oggraph_sb = load_const(oggraph, name="oggraph_sb")
            invgsz_sb = load_const(invgsz, name="invgsz_sb")
            idx16_sb = load_const(idx16, name="idx16_sb")
            WiT_sb = [load_const(WiT[l], name=f"WiT_sb{l}") for l in range(L)]
            WjT8_sb = [load_const(WjT8[l], name=f"WjT8_sb{l}") for l in range(L)]
            WeT8_sb = [load_const(WeT8[l], name=f"WeT8_sb{l}") for l in range(L)]
            brow_sb = [load_const(brow[l], name=f"brow_sb{l}") for l in range(L)]

            ident = const.tile([P, P], fp32, name="ident")
            make_identity(nc, ident[:])

            zeros_c = const.tile([P, 1], fp32, name="zeros_c")
            nc.vector.memset(zeros_c[:], 0.0)
            nc.const_aps.aps[(fp32, 0.0)] = zeros_c[:]
            eps_c = const.tile([P, 1], fp32, name="eps_c")
            nc.vector.memset(eps_c[:], BN_EPS)
            nc.const_aps.aps[(fp32, BN_EPS)] = eps_c[:]
            ones_c = const.tile([P, 1], fp32, name="ones_c")
            nc.vector.memset(ones_c[:], 1.0)
            nc.const_aps.aps[(fp32, 1.0)] = ones_c[:]

            # persistent state
            h_sb = state.tile([P, NTILES, H], fp32, name="h_sb")
            # rhs for the fused att||og matmul: slot0 = WeT, slot1 = afas_t
            afas_sb = state.tile([P, NTILES, 2, H2], fp8, name="afas_sb")
            h8_sb = state.tile([P, NTILES, H], fp8, name="h8_sb")
            nc.vector.memset(h8_sb[:], 0.0)

            for _rep in range(reps):
                hb = [[dram.tile([HNPAD, H], fp8, name=f"hb{l}_{hh}_{_rep}")
                       for hh in range(2)] for l in range(L)]
                hfull = [[dram.tile([HROWS, H], fp8, addr_space="Shared",
                                    name=f"hfull{l}_{hh}_{_rep}")
                          for hh in range(2)] for l in range(L)]
                stats_bounce = [dram.tile([1, H2], fp32, name=f"stats_bounce{l}_{_rep}")
                                for l in range(L)]
                stats_full = [dram.tile([1, H2], fp32, addr_space="Shared",
                                        name=f"stats_full{l}_{_rep}") for l in range(L)]
                g_bounce = dram.tile([NG, H], fp32, name=f"g_bounce{_rep}")
                g_full = dram.tile([NG, H], fp32, addr_space="Shared", name=f"g_full{_rep}")

                def snapshot_h8(t):
                    # fp8 snapshot of h tile (only real rows; padding
                    # partitions >=125 stay zero so ZROW=127 is a zero row)
                    nc.vector.tensor_copy(h8_sb[0:125, t, :], h_sb[0:125, t, :])

                def bounce_half(l, hh):
                    # h8_sb[:, t, :] rows (p) map to table row (t%HT)*P + p;
                    # walk the DRAM side p-major to match the SBUF AP order.
                    # Issued as soon as the half's 10 tiles are ready so the
                    # AllGather overlaps the rest of the phase.
                    dst = hb[l][hh]
                    nc.sync.dma_start(
                        dst[:].rearrange("(t p) c -> p t c", p=P),
                        h8_sb[:, hh * HT:(hh + 1) * HT, :])
                    if sim:
                        nc.sync.dma_start(hfull[l][hh][0:HNPAD, :], dst[:])
                    else:
                        nc.gpsimd.collective_compute(
                            "AllGather", OP.bypass, replica_groups=groups,
                            ins=[dst.opt()], outs=[hfull[l][hh].opt()])

                # ---------- phase H0: h = x @ W_emb^T + b_emb ----------
                for t in range(NTILES):
                    xTt = stream.tile([P, 2, P], bf16, tag="xTt", name="xTt")
                    nc.sync.dma_start(xTt[:], xT[:, :, t * P:(t + 1) * P])
                    hps = misc_ps([P, H], "hps")
                    nc.tensor.matmul(hps[:], xTt[:, 0, :],
                                     W_embT_sb[:, 0, :], start=True, stop=False)
                    nc.tensor.matmul(hps[:], xTt[:, 1, :],
                                     W_embT_sb[:, 1, :], start=False, stop=False)
                    nc.tensor.matmul(hps[:], onescol_sb[:1, :],
                                     b_embrow_sb[:1, :], start=False, stop=True)
                    nc.vector.tensor_copy(h_sb[:, t, :], hps[:])
                    snapshot_h8(t)
                    if t == HT - 1:
                        bounce_half(0, 0)
                bounce_half(0, 1)

                # ---------- layers ----------
                for l in range(L):
                    # --- per-tile rhs slot0 = WeT (const within layer) ---
                    for t in range(NTILES):
                        nc.gpsimd.tensor_copy(afas_sb[:, t, 0, :], WeT8_sb[l][:])

                    # --- node projections: afas (i-side) ---
                    for t in range(NTILES):
                        hT = work.tile([P, 2, P], bf16, tag="hT", bufs=2, name="hT")
                        for k in range(2):
                            tps = misc_ps([P, P], "tps")
                            nc.tensor.transpose(
                                tps[:], h_sb[:, t, k * P:(k + 1) * P], ident[:])
                            nc.vector.tensor_copy(hT[:, k, :], tps[:])
                        aps = misc_ps([P, H2], "aps")
                        nc.tensor.matmul(aps[:], hT[:, 0, :], WiT_sb[l][:, 0, :],
                                         start=True, stop=False)
                        nc.tensor.matmul(aps[:], hT[:, 1, :], WiT_sb[l][:, 1, :],
                                         start=False, stop=False)
                        nc.tensor.matmul(aps[:], onescol_sb[:1, :], brow_sb[l][:1, :],
                                         start=False, stop=True)
                        nc.vector.tensor_copy(afas_sb[:, t, 1, :], aps[:])

                    # --- BN stat accumulator: persistent PSUM bank, the
                    # per-tile sum/sumsq matmuls accumulate into it directly
                    # (no DVE adds) ---
                    stats_ps = stat_pool.tile([1, H2], fp32, name="stats_ps")
                    # zero via DVE once; all stats matmuls accumulate with
                    # start=False (two start=True groups sharing one bank
                    # would re-mark the 2KB zero-region and drop partials)
                    nc.vector.memset(stats_ps[:], 0.0)
                    stats_n = [0]

                    # --- edge passes: pass A = all-A chunks (table A only);
                    #     pass B = mixed + all-B chunks (tables A and B) ---
                    def process_chunks(t, c0, ncv, jmms):
                        """Process chunks [c0, c0+ncv) of tile t. jmms maps
                        local chunk index -> list of (gt_flat, byte_off)."""
                        b0 = base[t] + c0
                        att_t = stream.tile([P, ncv, 3, P], fp8, tag="att",
                                            name="att_t",
                                            padded_shape=[P, KTMAX, 3, P])
                        nc.sync.dma_start(att_t[:], attog[:, b0:b0 + ncv, :, :])

                        E_t = work.tile([P, ncv, H2], bf16, tag="E_t",
                                        name="E_t", bufs=2,
                                        padded_shape=[P, KTMAX, H2])
                        for i0 in range(0, ncv, 2):
                            n2 = min(2, ncv - i0)
                            pr = pre_pool.tile([P, 2, H2], fp32, name="pre")
                            for d in range(n2):
                                i = i0 + d
                                nc.tensor.matmul(
                                    pr[:, d, :], att_t[:, i, 0:2, :],
                                    afas_sb[:, t, :, :],
                                    start=True, stop=False, perf_mode=DR)
                                jl = jmms(i)
                                for w, (gflat, off) in enumerate(jl):
                                    nc.tensor.matmul(
                                        pr[:, d, :],
                                        gflat[:, off:off + 2 * P],
                                        WjT8_sb[l][:],
                                        start=False, stop=(w == len(jl) - 1),
                                        perf_mode=DRI)
                            nc.scalar.activation(
                                E_t[:, i0:i0 + n2, :],
                                pr[:, 0:n2, :] if n2 == 2 else pr[:, 0, :],
                                AF.Exp)

                        # u||v = ln(1+E); sigma = exp(-u); msg = sigma * v
                        uv_t = work.tile([P, ncv, H2], bf16, tag="uv_t",
                                         name="uv_t", bufs=2,
                                         padded_shape=[P, KTMAX, H2])
                        nc.scalar.activation(uv_t[:], E_t[:], AF.Ln, bias=1.0)
                        sg_t = work.tile([P, ncv, H], bf16, tag="sg_t",
                                         name="sg_t", bufs=2,
                                         padded_shape=[P, KTMAX, H])
                        nc.scalar.activation(sg_t[:], uv_t[:, :, 0:H],
                                             AF.Exp, scale=-1.0)
                        msg_t = work.tile([P, ncv, H], fp8, tag="msg_t",
                                          name="msg_t", bufs=2,
                                          padded_shape=[P, KTMAX, H])
                        nc.vector.tensor_tensor(out=msg_t[:], in0=sg_t[:],
                                                in1=uv_t[:, :, H:], op=OP.mult)

                        # paired fp8 DoubleRow scatter: two chunks per matmul
                        agg = agg_pool.tile([P, H], fp32, name="agg")
                        for i0 in range(0, ncv, 2):
                            if i0 + 2 <= ncv:
                                nc.tensor.matmul(
                                    agg[:], att_t[:, i0:i0 + 2, 2, :],
                                    msg_t[:, i0:i0 + 2, :],
                                    start=(i0 == 0), stop=(i0 + 2 == ncv),
                                    perf_mode=DR)
                            else:
                                nc.tensor.matmul(
                                    agg[:], att_t[:, i0, 2, :], msg_t[:, i0, :],
                                    start=(i0 == 0), stop=True)
                        nc.vector.scalar_tensor_tensor(
                            out=h_sb[:, t, :], in0=agg[:],
                            scalar=invdeg_sb[:, t:t + 1], in1=h_sb[:, t, :],
                            op0=OP.mult, op1=OP.add)

                    def gather_seg(src_full, col0, nch):
                        gt = stream.tile([P, 2 * KTMAX * P], fp8, tag="gt",
                                         name="gt")
                        gt_ap = gt[:, 0:2 * nch * P].rearrange(
                            "p (a e) -> p a e", a=2)
                        nc.gpsimd.dma_gather(
                            gt_ap, src_full[:],
                            idx16_sb[:, col0 * 8:(col0 + nch) * 8],
                            nch * P, nch * P, H, transpose=True)
                        return gt[:]

                    def tile_stats(t):
                        hsq = work.tile([P, H], fp32, tag="hsq", bufs=3,
                                        name="hsq")
                        nc.vector.tensor_tensor(out=hsq[:], in0=h_sb[:, t, :],
                                                in1=h_sb[:, t, :], op=OP.mult)
                        i = stats_n[0]
                        stats_n[0] += 1
                        nc.tensor.matmul(stats_ps[:, :H],
                                         onesmask_sb[:, t:t + 1],
                                         h_sb[:, t, :], start=False,
                                         stop=(i == NTILES - 1),
                                         skip_group_check=True)
                        nc.tensor.matmul(stats_ps[:, H:],
                                         onesmask_sb[:, t:t + 1],
                                         hsq[:], start=False,
                                         stop=(i == NTILES - 1),
                                         skip_group_check=True)

                    # First SPLIT tiles run their all-A chunks while the
                    # table-B AllGather is in flight; the rest run single-pass
                    # (one gather per table, all chunks, one h update).
                    SPLIT = 2
                    for t in range(SPLIT):
                        qa = QA[t]
                        if qa == 0:
                            continue
                        gA = gather_seg(hfull[l][0], segbase[t], qa)
                        process_chunks(t, 0, qa,
                                       lambda i, gA=gA: [(gA, i * 2 * P)])

                    for t in range(SPLIT, NTILES):
                        qa, qb, kt = QA[t], QB[t], KT[t]
                        nm = qb - qa
                        gA = gather_seg(hfull[l][0], segbase[t], qb) \
                            if qb > 0 else None
                        gB = gather_seg(hfull[l][1],
                                        segbase[t] + qb, kt - qa) \
                            if kt > qa else None

                        def jmms(i, gA=gA, gB=gB, qa=qa, qb=qb):
                            jl = []
                            if i < qb:
                                jl.append((gA, i * 2 * P))
                            if i >= qa:
                                jl.append((gB, (i - qa) * 2 * P))
                            return jl

                        process_chunks(t, 0, kt, jmms)
                        tile_stats(t)

                    # pass B of the split tiles
                    for t in range(SPLIT):
                        qa, qb, kt = QA[t], QB[t], KT[t]
                        nm = qb - qa
                        nb = kt - qa
                        if nb > 0:
                            gM = None
                            if nm > 0:
                                gM = gather_seg(hfull[l][0],
                                                segbase[t] + qa, nm)
                            gB = gather_seg(hfull[l][1],
                                            segbase[t] + qa + nm, nb)

                            def jmms(i, gM=gM, gB=gB, nm=nm):
                                jl = []
                                if i < nm:
                                    jl.append((gM, i * 2 * P))
                                jl.append((gB, i * 2 * P))
                                return jl

                            process_chunks(t, qa, nb, jmms)
                        tile_stats(t)

                    # --- BN: allreduce stats, apply affine ---
                    stats_acc = work.tile([1, H2], fp32, tag="stats_acc",
                                          name="stats_acc")
                    nc.vector.tensor_copy(stats_acc[:], stats_ps[:])
                    nc.sync.dma_start(stats_bounce[l][:], stats_acc[:])
                    if sim:
                        nc.sync.dma_start(stats_full[l][:], stats_bounce[l][:])
                    else:
                        nc.gpsimd.collective_compute(
                            "AllReduce", OP.add, replica_groups=groups,
                            ins=[stats_bounce[l].opt()],
                            outs=[stats_full[l].opt()])
                    statsr = work.tile([1, H2], fp32, tag="small", bufs=1, name="statsr")
                    nc.sync.dma_start(statsr[:], stats_full[l][:])
                    ab = work.tile([1, H2], fp32, tag="small", bufs=1, name="ab")
                    mu = work.tile([1, H], fp32, tag="small2", name="mu")
                    nc.vector.tensor_scalar_mul(mu[:], statsr[:, :H], 1.0 / N)
                    var = work.tile([1, H], fp32, tag="small2", name="var")
                    nc.vector.tensor_scalar_mul(var[:], statsr[:, H:], 1.0 / N)
                    musq = work.tile([1, H], fp32, tag="small2", name="musq")
                    nc.vector.tensor_tensor(out=musq[:], in0=mu[:], in1=mu[:],
                                            op=OP.mult)
                    nc.vector.tensor_tensor(out=var[:], in0=var[:], in1=musq[:],
                                            op=OP.subtract)
                    # A = gamma * exp(-0.5*ln(var+eps))  (no sqrt table needed)
                    lnv = work.tile([1, H], fp32, tag="small2", name="lnv")
                    nc.scalar.activation(lnv[:], var[:], AF.Ln, bias=BN_EPS)
                    rsd = work.tile([1, H], fp32, tag="small2", name="rsd")
                    nc.scalar.activation(rsd[:], lnv[:], AF.Exp, scale=-0.5)
                    nc.vector.tensor_tensor(out=ab[:, :H], in0=rsd[:],
                                            in1=gamma_sb[l][:], op=OP.mult)
                    nc.vector.tensor_tensor(out=ab[:, H:], in0=mu[:],
                                            in1=ab[:, :H], op=OP.mult)
                    nc.vector.tensor_tensor(out=ab[:, H:], in0=beta_sb[l][:],
                                            in1=ab[:, H:], op=OP.subtract)
                    abps = misc_ps([P, H2], "abps")
                    nc.tensor.matmul(abps[:], onescol_sb[:1, :], ab[:1, :],
                                     start=True, stop=True)
                    abb = work.tile([P, H2], fp32, tag="abb", bufs=1, name="abb")
                    nc.vector.tensor_copy(abb[:], abps[:])
                    for t in range(NTILES):
                        nc.vector.tensor_tensor(out=h_sb[:, t, :],
                                                in0=h_sb[:, t, :],
                                                in1=abb[:, :H], op=OP.mult)
                        nc.vector.tensor_tensor(out=h_sb[:, t, :],
                                                in0=h_sb[:, t, :],
                                                in1=abb[:, H:], op=OP.add)
                        if l + 1 < L:
                            snapshot_h8(t)
                            if t == HT - 1:
                                bounce_half(l + 1, 0)
                    if l + 1 < L:
                        bounce_half(l + 1, 1)

                # ---------- readout ----------
                gp = misc_ps([NG, H], "gp")
                for t in range(NTILES):
                    nc.tensor.matmul(gp[:], oggraph_sb[:, t, :], h_sb[:, t, :],
                                     start=(t == 0), stop=(t == NTILES - 1))
                gp_sb = work.tile([NG, H], fp32, tag="gp_sb", bufs=1, name="gp_sb")
                nc.vector.tensor_scalar(out=gp_sb[:], in0=gp[:],
                                        scalar1=invgsz_sb[:, :1], scalar2=None,
                                        op0=mybir.AluOpType.mult)
                nc.sync.dma_start(g_bounce[:], gp_sb[:])
                if sim:
                    nc.sync.dma_start(g_full[:], g_bounce[:])
                else:
                    nc.gpsimd.collective_compute(
                        "AllReduce", OP.add, replica_groups=groups,
                        ins=[g_bounce.opt()], outs=[g_full.opt()])
                gr = work.tile([NG, H], fp32, tag="gr", bufs=1, name="gr")
                nc.sync.dma_start(gr[:], g_full[:])
                ge = work.tile([NG, H], fp32, tag="ge", bufs=1, name="ge")
                nc.scalar.activation(ge[:], gr[:], AF.Exp)
                spg = work.tile([NG, H], fp32, tag="spg", bufs=1, name="spg")
                nc.scalar.activation(spg[:], ge[:], AF.Ln, bias=1.0)
                spgT = work.tile([P, 2, NG], bf16, tag="spgT", bufs=1, name="spgT")
                for k in range(2):
                    tp = misc_ps([P, NG], "tp")
                    nc.tensor.transpose(tp[:], spg[:, k * P:(k + 1) * P],
                                        ident[:NG, :NG])
                    nc.scalar.activation(spgT[:, k, :], tp[:], AF.Copy)
                ops_ = misc_ps([NG, H], "ops_")
                nc.tensor.matmul(ops_[:], spgT[:, 0, :], W_fcT_sb[:, 0, :],
                                 start=True, stop=False)
                nc.tensor.matmul(ops_[:], spgT[:, 1, :], W_fcT_sb[:, 1, :],
                                 start=False, stop=False)
                nc.tensor.matmul(ops_[:], onescol_sb[:1, :NG], b_fcrow_sb[:1, :],
                                 start=False, stop=True)
                oe = work.tile([NG, H], fp32, tag="oe", bufs=1, name="oe")
                nc.scalar.activation(oe[:], ops_[:], AF.Exp)
                out_sb = work.tile([NG, H], fp32, tag="out_sb", bufs=1, name="out_sb")
                nc.scalar.activation(out_sb[:], oe[:], AF.Ln, bias=1.0)
                nc.sync.dma_start(out[:], out_sb[:])

    nc.compile()
    return nc


def kernel(**inputs):
    from concourse import bass_utils

    cfg, shared, per_core = _prep(**inputs)
    nc = _build(cfg)

    in_maps = []
    for c in range(NCORES):
        m = dict(shared)
        m.update(per_core[c])
        in_maps.append(m)

    res = bass_utils.run_bass_kernel_spmd(
        nc, in_maps, core_ids=list(range(NCORES)))
    return np.asarray(res.results[0]["out"], np.float32)


if __name__ == "__main__":
    data = np.load("/root/problem/ref_data.npz")
    inputs = {k: data[k] for k in data.files if k != "expected"}
    got = kernel(**inputs)
    exp = data["expected"]
    err = np.abs(got - exp).max() / max(np.abs(exp).max(), 1e-9)
    fro = np.linalg.norm(got - exp) / np.linalg.norm(exp)
    print("max abs rel err:", err, "fro:", fro)
